# revision 1
# baseline (speedup 1.0000x reference)
"""DistanceWeightedAttention Trainium2 kernel (8 NeuronCores, SPMD).

Strategy (src-partitioned, per sharding hint):
  - Sort edges by src; cut into 8 spans at row boundaries -> each core owns a
    disjoint range of query rows and ALL edges of those rows (segment softmax
    is core-local; final outputs are disjoint row blocks; no collectives).
  - Within a core, greedy-pack rows into "bins" of <=128 rows and <=EPB edges.
    Each bin is CPB chunks of 128 edge slots (padded; pad edges get an
    additive -80 mask so exp() -> ~0 and they contribute nothing).
  - Device pipeline per core:
      * project K,V -> KV table in DRAM [NKV_PAD, 256]; Q -> Qtable [R, 128]
        (bias folded in via rank-1 matmul into PSUM).
      * per 4-bin group: dma_gather Qe rows + KV rows per edge (SWDGE).
      * per 128-edge chunk: DVE mul + 32-group reduce -> scores[e,4];
        mul rbf; ACT exp(+mask bias); DVE bcast-mul exp*Ve -> wv;
        GPSIMD is_equal(iota, srcrel) -> one-hot^T [e,r];
        PE matmul onehotT.T @ [exp | wv] accumulated over the bin's chunks
        in PSUM -> [r, 4+128] = segment sums (denom | outU).
      * per bin: recip(denom+1e-8); outN = outU * recip (bcast over 32);
        PE transpose; outN^T @ Wo -> out rows (bo added on host).
  - Softmax uses the unstable form exp(s)/(sum exp(s) + 1e-8): scores are
    O(5) here so no overflow, and vs the reference's max(0, segmax) form the
    relative deviation is < 1e-8 (denom >= exp(m)).
"""

import math
import sys

import numpy as np

sys.path.insert(0, "/opt/trn_rl_repo")

HIDDEN = 128
HEADS = 4
HD = 32
SCALE = float(np.sqrt(HD))
NCORES = 8
CPB = 5              # chunks per bin
CHUNK = 128
EPB = CPB * CHUNK    # edge slots per bin
GROUP_BINS = 4       # bins per dma_gather group
GEDGES = GROUP_BINS * EPB   # 2560 edges per gather group
MASK_PAD = -80.0

_PROG_CACHE = {}


def _pack_core(rlo, rhi, deg, e_starts):
    """Greedy-pack rows [rlo, rhi) into bins (<=128 rows, <=EPB edges).

    Returns list of bins: (row_start, n_rows, edge_start, n_edges) where
    edge_start indexes the globally src-sorted edge array.
    """
    bins = []
    b_r0 = rlo
    b_rows = 0
    b_edges = 0
    for r in range(rlo, rhi):
        d = int(deg[r])
        if b_rows == 127 or (b_edges + d > EPB and b_rows > 0):
            bins.append((b_r0, b_rows, int(e_starts[b_r0]), b_edges))
            b_r0 = r
            b_rows = 0
            b_edges = 0
        b_rows += 1
        b_edges += d
    if b_rows > 0:
        bins.append((b_r0, b_rows, int(e_starts[b_r0]), b_edges))
    return bins


def _build_program(nbins, nkv_pad, r_total):
    import concourse.bass as bass
    import concourse.bacc as bacc
    import concourse.tile as tile
    from concourse import mybir

    f32 = mybir.dt.float32
    i16 = mybir.dt.int16
    nchunk = nbins * CPB
    ngroups = nbins // GROUP_BINS
    nkv_tiles = nkv_pad // 128
    KSLAB = 16           # kv proj tiles per slab load
    QSLAB = 8            # q proj bins per slab

    nc = bacc.Bacc("TRN2", target_bir_lowering=False, debug=False,
                   num_devices=NCORES)

    # ---- I/O -------------------------------------------------------------
    t_qT = nc.dram_tensor("qT", [128, r_total], f32, kind="ExternalInput")
    t_kT = nc.dram_tensor("kT", [128, nkv_pad], f32, kind="ExternalInput")
    t_vT = nc.dram_tensor("vT", [128, nkv_pad], f32, kind="ExternalInput")
    t_Wq = nc.dram_tensor("Wq", [128, 128], f32, kind="ExternalInput")
    t_Wk = nc.dram_tensor("Wk", [128, 128], f32, kind="ExternalInput")
    t_Wv = nc.dram_tensor("Wv", [128, 128], f32, kind="ExternalInput")
    t_Wo = nc.dram_tensor("Wo", [128, 128], f32, kind="ExternalInput")
    t_bq = nc.dram_tensor("bq", [1, 128], f32, kind="ExternalInput")
    t_bk = nc.dram_tensor("bk", [1, 128], f32, kind="ExternalInput")
    t_bv = nc.dram_tensor("bv", [1, 128], f32, kind="ExternalInput")
    t_ones = nc.dram_tensor("ones1", [1, 128], f32, kind="ExternalInput")
    t_iota = nc.dram_tensor("iota", [128, 128], f32, kind="ExternalInput")
    t_ident = nc.dram_tensor("ident", [128, 128], f32, kind="ExternalInput")
    t_srcrel = nc.dram_tensor("srcrel", [128, nchunk], f32, kind="ExternalInput")
    t_rbf = nc.dram_tensor("rbf", [128, nchunk * HEADS], f32, kind="ExternalInput")
    t_qidx = nc.dram_tensor("qidx", [128, nchunk * 8], i16, kind="ExternalInput")
    t_didx = nc.dram_tensor("didx", [128, nchunk * 8], i16, kind="ExternalInput")
    t_out = nc.dram_tensor("out", [128, r_total], f32, kind="ExternalOutput")

    with tile.TileContext(nc) as tc:
        with (
            tc.tile_pool(name="const", bufs=1) as constp,
            tc.tile_pool(name="slab", bufs=2) as slabp,
            tc.tile_pool(name="work", bufs=2) as work,
            tc.tile_pool(name="qe", bufs=2) as qep,
            tc.tile_pool(name="kve", bufs=2) as kvep,
            tc.tile_pool(name="sc", bufs=6) as scp,
            tc.tile_pool(name="wvp", bufs=6) as wvp,
            tc.tile_pool(name="oh", bufs=6) as ohp,
            tc.tile_pool(name="fin", bufs=4) as finp,
            tc.tile_pool(name="ps", bufs=2, space="PSUM") as psp,
            tc.tile_pool(name="tp", bufs=1, space="PSUM") as tpp,
            tc.tile_pool(name="binps", bufs=2, space="PSUM") as binpsp,
            tc.tile_pool(name="dram", bufs=1, space="DRAM") as dramp,
        ):
            # resident constants
            Wq = constp.tile([128, 128], f32, tag="Wq")
            Wk = constp.tile([128, 128], f32, tag="Wk")
            Wv = constp.tile([128, 128], f32, tag="Wv")
            Wo = constp.tile([128, 128], f32, tag="Wo")
            bq = constp.tile([1, 128], f32, tag="bq")
            bk = constp.tile([1, 128], f32, tag="bk")
            bv = constp.tile([1, 128], f32, tag="bv")
            ones = constp.tile([1, 128], f32, tag="ones")
            iota = constp.tile([128, 128], f32, tag="iota")
            ident = constp.tile([128, 128], f32, tag="ident")
            srcrel = constp.tile([128, nchunk], f32, tag="srcrel")
            qidx = constp.tile([128, nchunk * 8], i16, tag="qidx")
            didx = constp.tile([128, nchunk * 8], i16, tag="didx")
            rbf_c = constp.tile([128, nchunk * HEADS], f32, tag="rbfc")
            nc.sync.dma_start(Wq[:], t_Wq[:])
            nc.sync.dma_start(Wk[:], t_Wk[:])
            nc.sync.dma_start(Wv[:], t_Wv[:])
            nc.sync.dma_start(Wo[:], t_Wo[:])
            nc.sync.dma_start(bq[:], t_bq[:])
            nc.sync.dma_start(bk[:], t_bk[:])
            nc.sync.dma_start(bv[:], t_bv[:])
            nc.sync.dma_start(ones[:], t_ones[:])
            nc.sync.dma_start(iota[:], t_iota[:])
            nc.sync.dma_start(ident[:], t_ident[:])
            nc.scalar.dma_start(srcrel[:], t_srcrel[:])
            nc.scalar.dma_start(qidx[:], t_qidx[:])
            nc.scalar.dma_start(didx[:], t_didx[:])
            nc.scalar.dma_start(rbf_c[:], t_rbf[:])
            rbf_v = rbf_c[:].rearrange("p (c f) -> p c f", f=HEADS)

            # DRAM tables
            kvtab = dramp.tile([nkv_pad, 256], f32, tag="kvtab")
            qtab = dramp.tile([r_total, 128], f32, tag="qtab")

            # ---- K/V projection -> kvtab (slab-batched) ------------------
            for s0 in range(0, nkv_tiles, KSLAB):
                nt = min(KSLAB, nkv_tiles - s0)
                ksl = slabp.tile([128, KSLAB * 128], f32, tag="ksl")
                vsl = slabp.tile([128, KSLAB * 128], f32, tag="vsl")
                nc.sync.dma_start(ksl[:, 0:nt * 128],
                                  t_kT[:, s0 * 128:(s0 + nt) * 128])
                nc.scalar.dma_start(vsl[:, 0:nt * 128],
                                    t_vT[:, s0 * 128:(s0 + nt) * 128])
                for g0 in range(0, nt, 2):
                    kvps = psp.tile([128, 512], f32, tag="mm")
                    for i in range(2):
                        t = g0 + i
                        lo = i * 256
                        nc.tensor.matmul(kvps[:, lo:lo + 128], ones[:], bk[:],
                                         start=True, stop=False)
                        nc.tensor.matmul(kvps[:, lo:lo + 128],
                                         ksl[:, t * 128:(t + 1) * 128], Wk[:],
                                         start=False, stop=True)
                        nc.tensor.matmul(kvps[:, lo + 128:lo + 256], ones[:],
                                         bv[:], start=True, stop=False)
                        nc.tensor.matmul(kvps[:, lo + 128:lo + 256],
                                         vsl[:, t * 128:(t + 1) * 128], Wv[:],
                                         start=False, stop=True)
                    kvsb = work.tile([128, 512], f32, tag="kvsb")
                    nc.scalar.copy(kvsb[:], kvps[:])
                    nc.gpsimd.dma_start(
                        kvtab[(s0 + g0) * 128:(s0 + g0 + 2) * 128, :].rearrange(
                            "(t p) f -> p t f", p=128),
                        kvsb[:].rearrange("p (t f) -> p t f", f=256))

            # ---- Q projection -> qtab (slab-batched) ---------------------
            assert nbins % QSLAB == 0
            for b0 in range(0, nbins, QSLAB):
                qsl = slabp.tile([128, QSLAB * 128], f32, tag="qsl")
                nc.sync.dma_start(qsl[:], t_qT[:, b0 * 128:(b0 + QSLAB) * 128])
                for g0 in range(0, QSLAB, 4):
                    qps = psp.tile([128, 512], f32, tag="mm")
                    for i in range(4):
                        t = g0 + i
                        lo = i * 128
                        nc.tensor.matmul(qps[:, lo:lo + 128], ones[:], bq[:],
                                         start=True, stop=False)
                        nc.tensor.matmul(qps[:, lo:lo + 128],
                                         qsl[:, t * 128:(t + 1) * 128], Wq[:],
                                         start=False, stop=True)
                    qsb = work.tile([128, 512], f32, tag="qsb")
                    nc.scalar.copy(qsb[:], qps[:])
                    nc.gpsimd.dma_start(
                        qtab[(b0 + g0) * 128:(b0 + g0 + 4) * 128, :].rearrange(
                            "(t p) f -> p t f", p=128),
                        qsb[:].rearrange("p (t f) -> p t f", f=128))

            # ---- main edge loop -----------------------------------------
            for G in range(ngroups):
                qe = qep.tile([128, GEDGES // 128, 128], f32, tag="qe")
                kve = kvep.tile([128, GEDGES // 128, 256], f32, tag="kve")
                i0 = G * (GEDGES // 16)
                nc.gpsimd.dma_gather(
                    out_ap=qe[:], in_ap=qtab[:],
                    idxs_ap=qidx[:, i0:i0 + GEDGES // 16],
                    num_idxs=GEDGES, num_idxs_reg=GEDGES, elem_size=128,
                    single_packet=False,
                )
                nc.gpsimd.dma_gather(
                    out_ap=kve[:], in_ap=kvtab[:],
                    idxs_ap=didx[:, i0:i0 + GEDGES // 16],
                    num_idxs=GEDGES, num_idxs_reg=GEDGES, elem_size=256,
                    single_packet=False,
                )
                ops4 = None
                for j in range(GROUP_BINS):
                    b = G * GROUP_BINS + j
                    # pass 1: scores for the bin's CPB chunks
                    scb = scp.tile([128, CPB * HEADS], f32, tag="scb")
                    for k in range(CPB):
                        cc = j * CPB + k
                        c = b * CPB + k
                        prod = scp.tile([128, 128], f32, tag="prod")
                        nc.gpsimd.tensor_tensor(
                            prod[:], qe[:, cc, :], kve[:, cc, 0:128],
                            op=mybir.AluOpType.mult)
                        sc4 = scp.tile([128, HEADS], f32, tag="sc4")
                        nc.vector.tensor_reduce(
                            sc4[:], prod[:].rearrange("p (h d) -> p h d", d=HD),
                            axis=mybir.AxisListType.X, op=mybir.AluOpType.add)
                        nc.vector.tensor_tensor(
                            scb[:, k * HEADS:(k + 1) * HEADS], sc4[:],
                            rbf_v[:, c, :], op=mybir.AluOpType.mult)
                    exps = scp.tile([128, CPB * HEADS], f32, tag="exps")
                    nc.scalar.activation(
                        exps[:], scb[:], mybir.ActivationFunctionType.Exp)
                    # pass 2: wv, one-hot, segment-sum matmuls
                    bpd = binpsp.tile([128, 4], f32, tag="bpd")
                    bps = binpsp.tile([128, 128], f32, tag="bps")
                    for k in range(CPB):
                        cc = j * CPB + k
                        c = b * CPB + k
                        oh = ohp.tile([128, 128], f32, tag="oh")
                        nc.vector.tensor_scalar(
                            oh[:], iota[:], srcrel[:, c:c + 1], None,
                            op0=mybir.AluOpType.is_equal)
                        wv = wvp.tile([128, 128], f32, tag="wv")
                        ebc = exps[:, k * HEADS:(k + 1) * HEADS].unsqueeze(
                            2).broadcast_to([128, HEADS, HD])
                        nc.vector.tensor_tensor(
                            wv[:].rearrange("p (h d) -> p h d", d=HD),
                            ebc,
                            kve[:, cc, 128:256].rearrange(
                                "p (h d) -> p h d", d=HD),
                            op=mybir.AluOpType.mult)
                        nc.tensor.matmul(
                            bpd[:], oh[:],
                            exps[:, k * HEADS:(k + 1) * HEADS],
                            start=(k == 0), stop=(k == CPB - 1))
                        nc.tensor.matmul(
                            bps[:], oh[:], wv[:],
                            start=(k == 0), stop=(k == CPB - 1))
                    # bin epilogue
                    den = finp.tile([128, HEADS], f32, tag="den")
                    nc.vector.tensor_scalar_add(den[:], bpd[:], 1e-8)
                    rec = finp.tile([128, HEADS], f32, tag="rec")
                    nc.vector.reciprocal(rec[:], den[:])
                    onrm = finp.tile([128, 128], f32, tag="onrm")
                    rbc = rec[:].unsqueeze(2).broadcast_to([128, HEADS, HD])
                    nc.vector.tensor_tensor(
                        onrm[:].rearrange("p (h d) -> p h d", d=HD),
                        bps[:].rearrange("p (h d) -> p h d", d=HD),
                        rbc, op=mybir.AluOpType.mult)
                    tps = tpp.tile([128, 128], f32, tag="tps")
                    nc.tensor.transpose(tps[:], onrm[:], ident[:])
                    onrmT = finp.tile([128, 128], f32, tag="onrmT")
                    nc.scalar.copy(onrmT[:], tps[:])
                    if j == 0:
                        ops4 = psp.tile([128, 512], f32, tag="mm")
                    nc.tensor.matmul(ops4[:, j * 128:(j + 1) * 128],
                                     onrmT[:], Wo[:], start=True, stop=True)
                osb = finp.tile([128, 512], f32, tag="osb")
                nc.scalar.copy(osb[:], ops4[:])
                nc.sync.dma_start(
                    t_out[:, G * 512:(G + 1) * 512], osb[:])

    nc.compile()
    return nc


def _wrap16(idx, n_slots):
    """[n] int array -> [128, n/16] int16 wrapped (i at [i%16, i//16]), tiled x8."""
    w = np.zeros((16, n_slots // 16), dtype=np.int16)
    w[:, :] = idx.astype(np.int16).reshape(n_slots // 16, 16).T
    return np.tile(w, (8, 1))


def kernel(**inputs):
    query = np.asarray(inputs["query"], np.float32)
    key_in = np.asarray(inputs["key_in"], np.float32)
    value_in = np.asarray(inputs["value_in"], np.float32)
    src = np.asarray(inputs["src"]).astype(np.int64)
    dst = np.asarray(inputs["dst"]).astype(np.int64)
    ea = np.asarray(inputs["edge_attr"], np.float32).reshape(-1)
    Wq = np.asarray(inputs["Wq"], np.float32)
    Wk = np.asarray(inputs["Wk"], np.float32)
    Wv = np.asarray(inputs["Wv"], np.float32)
    Wo = np.asarray(inputs["Wo"], np.float32)
    bq = np.asarray(inputs["bq"], np.float32)
    bk = np.asarray(inputs["bk"], np.float32)
    bv = np.asarray(inputs["bv"], np.float32)
    bo = np.asarray(inputs["bo"], np.float32)
    rbf_gamma = np.asarray(inputs["rbf_gamma"], np.float32)

    nq = query.shape[0]
    nkv = key_in.shape[0]
    E = src.shape[0]
    nkv_pad = ((nkv + 511) // 512) * 512

    gamma = np.maximum(rbf_gamma, np.float32(1e-8))
    rbf_all = (np.exp(-(gamma[None, :].astype(np.float32))
                      * (ea[:, None] ** 2)) / np.float32(SCALE)).astype(np.float32)

    order = np.argsort(src, kind="stable")
    ssrc = src[order]
    sdst = dst[order]
    srbf = rbf_all[order]

    deg = np.bincount(src, minlength=nq).astype(np.int64)
    e_starts = np.zeros(nq + 1, dtype=np.int64)
    np.cumsum(deg, out=e_starts[1:])

    # core cuts at row boundaries
    cuts = [0]
    for c in range(1, NCORES):
        p = c * (E // NCORES)
        while p < E and ssrc[p] == ssrc[p - 1]:
            p += 1
        cuts.append(int(p))
    cuts.append(E)
    rlo = [0] * NCORES
    rhi = [0] * NCORES
    for c in range(NCORES):
        if c == 0:
            rlo[c] = 0
        else:
            rlo[c] = int(ssrc[cuts[c]]) if cuts[c] < E else nq
    for c in range(NCORES):
        rhi[c] = rlo[c + 1] if c < NCORES - 1 else nq

    core_bins = []
    nb_max = 0
    for c in range(NCORES):
        bins = _pack_core(rlo[c], rhi[c], deg, e_starts)
        core_bins.append(bins)
        nb_max = max(nb_max, len(bins))
    nbins = ((nb_max + 7) // 8) * 8
    r_total = nbins * 128
    nchunk = nbins * CPB

    key = (nbins, nkv_pad, r_total)
    if key not in _PROG_CACHE:
        _PROG_CACHE[key] = _build_program(nbins, nkv_pad, r_total)
    nc = _PROG_CACHE[key]

    # shared tensors
    kT_pad = np.zeros((128, nkv_pad), np.float32)
    kT_pad[:, :nkv] = key_in.T
    vT_pad = np.zeros((128, nkv_pad), np.float32)
    vT_pad[:, :nkv] = value_in.T
    iota_t = np.broadcast_to(np.arange(128, dtype=np.float32), (128, 128)).copy()
    ident_t = np.eye(128, dtype=np.float32)
    ones_t = np.ones((1, 128), np.float32)

    in_maps = []
    unpack = []
    for c in range(NCORES):
        bins = core_bins[c]
        qT = np.zeros((128, r_total), np.float32)
        srcrel = np.full((128, nchunk), np.float32(127.0), np.float32)
        rbf_a = np.zeros((128, nchunk, HEADS), np.float32)
        qidx_a = np.zeros(nchunk * 128, np.int64)
        didx_a = np.zeros(nchunk * 128, np.int64)
        rows_glob = np.zeros(r_total, np.int64) - 1

        for b, (r0, nr, e0, ne) in enumerate(bins):
            qT[:, b * 128:b * 128 + nr] = query[r0:r0 + nr].T
            rows_glob[b * 128:b * 128 + nr] = np.arange(r0, r0 + nr)
            # edges of this bin occupy sorted positions [e0, e0+ne)
            pos = b * EPB + np.arange(ne)
            erel = ssrc[e0:e0 + ne] - r0          # row-in-bin (rows contiguous)
            # srcrel layout: [128 part, nchunk] column c = chunk's 128 edges
            ch = pos // 128
            sl = pos % 128
            srcrel[sl, ch] = erel.astype(np.float32)
            rbf_a[sl, ch, :] = srbf[e0:e0 + ne]
            qidx_a[pos] = b * 128 + erel
            didx_a[pos] = sdst[e0:e0 + ne]

        in_maps.append({
            "qT": qT, "kT": kT_pad, "vT": vT_pad,
            "Wq": Wq, "Wk": Wk, "Wv": Wv, "Wo": Wo,
            "bq": bq.reshape(1, 128), "bk": bk.reshape(1, 128),
            "bv": bv.reshape(1, 128),
            "ones1": ones_t, "iota": iota_t, "ident": ident_t,
            "srcrel": srcrel, "rbf": rbf_a.reshape(128, -1),
            "qidx": _wrap16(qidx_a, nchunk * 128),
            "didx": _wrap16(didx_a, nchunk * 128),
        })
        unpack.append(rows_glob)

    from concourse.bass_utils import run_bass_kernel_spmd
    g = globals()
    g["LAST_NC"] = nc
    g["LAST_INMAPS"] = in_maps
    res = run_bass_kernel_spmd(nc, in_maps, list(range(NCORES)),
                               trace=g.get("TRACE", False))
    g["LAST_RESULTS"] = res

    out = np.zeros((nq, HIDDEN), np.float32)
    for c in range(NCORES):
        o = np.asarray(res.results[c]["out"])  # [128, nbins*128] part-major
        o = o.reshape(128, -1, 128).transpose(1, 0, 2).reshape(-1, 128)
        valid = unpack[c] >= 0
        out[unpack[c][valid]] = o[valid]
    out += bo[None, :]
    return out



# revision 5
# speedup vs baseline: 1.7173x; 1.7173x over previous
"""DistanceWeightedAttention Trainium2 kernel (8 NeuronCores, SPMD).

Strategy (degree-sorted row layout):
  - Rows (query nodes) with deg>0 are sorted by degree and dealt round-robin
    to the 8 cores, so every core sees an identical degree profile and the
    SPMD program (one bass module for all cores) has a common bin template.
  - A bin = 128 rows (partition dim) x D slots (free dim), D = max degree in
    the bin; degree sorting makes padding negligible. Edge (row j, slot s) of
    bin b sits at gather position (slotbase_b + s)*128 + j, so a single
    SWDGE gather of [K|V] fp16 rows (512B descriptors -- the cost-model
    sweet spot) lands kve[j, slot, 0:256] with partition = row.
  - Per bin, on-device:
      prod = K_e * q_row (DVE fp16, row broadcast over slots, 2x mode)
      scores = pairwise-add cascade 32->16->8->4->2->1 (all fp16 TT, 2x)
      scores = scores * rbf + mask  (pads get -80 -> exp == 0 in fp16)
      exps = ACT Exp; den = sum_s exps via PE identity-matmul accumulation
      rec = 1/(den+1e-8); exn = exps*rec; pair-duplicated exps2 so the
      wv multiply keeps a stride-1 last dim (DVE 2x mode)
      wv = V_e * probs; outU^T accumulated with matmul(lhsT=wv, rhs=I)
      out = outU^T.T @ Wo via one matmul per bin (bo added on host)
  - K/V/Q projections are computed on device in fp16 (biases as rank-1
    matmuls); kvtab ([K|V] per dst node) is staged in DRAM fp16.
  - Softmax uses the unstable form exp(s)/(sum exp(s)+1e-8); scores are O(1)
    here so this matches the reference's max(0,segmax) form to ~1e-7.
"""

import sys

import numpy as np

sys.path.insert(0, "/opt/trn_rl_repo")

HIDDEN = 128
HEADS = 4
HD = 32
SCALE = float(np.sqrt(HD))
NCORES = 8
MASK_PAD = -80.0
MAX_GROUP_SLOTS = 40      # slot-chunks per gather group cap (SBUF)
MAX_GROUP_BINS = 4

_PROG_CACHE = {}


def _build_program(nkv_pad, nbins, D_list, groups, total_slots):
    import concourse.bass as bass
    import concourse.bacc as bacc
    import concourse.tile as tile
    from concourse import mybir

    f32 = mybir.dt.float32
    f16 = mybir.dt.float16
    i16 = mybir.dt.int16
    nkv_tiles = nkv_pad // 128
    KSLAB = 16
    maxD = max(D_list)
    max_gslots = max(sum(D_list[b] for b in g) for g in groups)
    slotbase = np.concatenate([[0], np.cumsum(D_list)]).astype(int)

    nc = bacc.Bacc("TRN2", target_bir_lowering=False, debug=False,
                   num_devices=NCORES)

    t_qT = nc.dram_tensor("qT", [128, nbins * 128], f16, kind="ExternalInput")
    t_kT = nc.dram_tensor("kT", [128, nkv_pad], f16, kind="ExternalInput")
    t_vT = nc.dram_tensor("vT", [128, nkv_pad], f16, kind="ExternalInput")
    t_Wq = nc.dram_tensor("Wq", [128, 128], f16, kind="ExternalInput")
    t_Wk = nc.dram_tensor("Wk", [128, 128], f16, kind="ExternalInput")
    t_Wv = nc.dram_tensor("Wv", [128, 128], f16, kind="ExternalInput")
    t_Wo = nc.dram_tensor("Wo", [128, 128], f16, kind="ExternalInput")
    t_bq = nc.dram_tensor("bq", [1, 128], f16, kind="ExternalInput")
    t_bk = nc.dram_tensor("bk", [1, 128], f16, kind="ExternalInput")
    t_bv = nc.dram_tensor("bv", [1, 128], f16, kind="ExternalInput")
    t_ones = nc.dram_tensor("ones1", [1, 128], f16, kind="ExternalInput")
    t_ident = nc.dram_tensor("ident", [128, 128], f16, kind="ExternalInput")
    t_rbfm = nc.dram_tensor("rbfm", [128, total_slots * HEADS], f16,
                            kind="ExternalInput")
    t_mask = nc.dram_tensor("mask", [128, total_slots * HEADS], f16,
                            kind="ExternalInput")
    t_didx = nc.dram_tensor("didx", [128, total_slots * 8], i16,
                            kind="ExternalInput")
    t_out = nc.dram_tensor("out", [128, nbins * 128], f16,
                           kind="ExternalOutput")

    with tile.TileContext(nc) as tc:
        with (
            tc.tile_pool(name="const", bufs=1) as constp,
            tc.tile_pool(name="slab", bufs=2) as slabp,
            tc.tile_pool(name="work", bufs=3) as work,
            tc.tile_pool(name="kve", bufs=2) as kvep,
            tc.tile_pool(name="edge", bufs=3) as edgep,
            tc.tile_pool(name="sm", bufs=4) as smp,
            tc.tile_pool(name="fin", bufs=3) as finp,
            tc.tile_pool(name="ps", bufs=2, space="PSUM") as psp,
            tc.tile_pool(name="dps", bufs=2, space="PSUM") as dpsp,
            tc.tile_pool(name="ops", bufs=2, space="PSUM") as opsp,
            tc.tile_pool(name="fps", bufs=2, space="PSUM") as fpsp,
            tc.tile_pool(name="dram", bufs=1, space="DRAM") as dramp,
        ):
            Wq = constp.tile([128, 128], f16, tag="Wq")
            Wk = constp.tile([128, 128], f16, tag="Wk")
            Wv = constp.tile([128, 128], f16, tag="Wv")
            Wo = constp.tile([128, 128], f16, tag="Wo")
            bq = constp.tile([1, 128], f16, tag="bq")
            bk = constp.tile([1, 128], f16, tag="bk")
            bv = constp.tile([1, 128], f16, tag="bv")
            ones = constp.tile([1, 128], f16, tag="ones")
            ident = constp.tile([128, 128], f16, tag="ident")
            rbfm = constp.tile([128, total_slots * HEADS], f16, tag="rbfm")
            maskb = constp.tile([128, total_slots * HEADS], f16, tag="maskb")
            didx = constp.tile([128, total_slots * 8], i16, tag="didx")
            qT = constp.tile([128, nbins * 128], f16, tag="qT")
            qproj = constp.tile([128, nbins * 128], f16, tag="qproj")
            nc.sync.dma_start(Wq[:], t_Wq[:])
            nc.sync.dma_start(Wk[:], t_Wk[:])
            nc.sync.dma_start(Wv[:], t_Wv[:])
            nc.sync.dma_start(Wo[:], t_Wo[:])
            nc.sync.dma_start(bq[:], t_bq[:])
            nc.sync.dma_start(bk[:], t_bk[:])
            nc.sync.dma_start(bv[:], t_bv[:])
            nc.sync.dma_start(ones[:], t_ones[:])
            nc.sync.dma_start(ident[:], t_ident[:])
            nc.scalar.dma_start(rbfm[:], t_rbfm[:])
            nc.scalar.dma_start(maskb[:], t_mask[:])
            nc.scalar.dma_start(didx[:], t_didx[:])
            nc.sync.dma_start(qT[:], t_qT[:])
            rbf_v = rbfm[:].rearrange("p (s h) -> p s h", h=HEADS)
            mask_v = maskb[:].rearrange("p (s h) -> p s h", h=HEADS)

            kvtab = dramp.tile([nkv_pad, 256], f16, tag="kvtab")

            # ---- K/V projection -> kvtab fp16 ---------------------------
            for s0 in range(0, nkv_tiles, KSLAB):
                nt = min(KSLAB, nkv_tiles - s0)
                ksl = slabp.tile([128, KSLAB * 128], f16, tag="ksl")
                vsl = slabp.tile([128, KSLAB * 128], f16, tag="vsl")
                nc.sync.dma_start(ksl[:, 0:nt * 128],
                                  t_kT[:, s0 * 128:(s0 + nt) * 128])
                nc.scalar.dma_start(vsl[:, 0:nt * 128],
                                    t_vT[:, s0 * 128:(s0 + nt) * 128])
                for g0 in range(0, nt, 4):
                    ng = min(4, nt - g0)
                    kvsb = work.tile([128, 1024], f16, tag="kvsb")
                    for h0 in range(0, ng, 2):
                        kvps = psp.tile([128, 512], f32, tag="mm")
                        for i in range(2):
                            t = g0 + h0 + i
                            if t >= nt:
                                continue
                            lo = i * 256
                            nc.tensor.matmul(kvps[:, lo:lo + 128], ones[:],
                                             bk[:], start=True, stop=False)
                            nc.tensor.matmul(kvps[:, lo:lo + 128],
                                             ksl[:, t * 128:(t + 1) * 128],
                                             Wk[:], start=False, stop=True)
                            nc.tensor.matmul(kvps[:, lo + 128:lo + 256],
                                             ones[:], bv[:],
                                             start=True, stop=False)
                            nc.tensor.matmul(kvps[:, lo + 128:lo + 256],
                                             vsl[:, t * 128:(t + 1) * 128],
                                             Wv[:], start=False, stop=True)
                        nc.scalar.copy(kvsb[:, h0 * 256:h0 * 256 + 512],
                                       kvps[:])
                    nc.gpsimd.dma_start(
                        kvtab[(s0 + g0) * 128:(s0 + g0 + ng) * 128, :]
                        .rearrange("(t p) f -> p t f", p=128),
                        kvsb[:, 0:ng * 256].rearrange(
                            "p (t f) -> p t f", f=256))

            # ---- Q projection -> qproj fp16 (SBUF resident) -------------
            for b0 in range(0, nbins, 4):
                nb = min(4, nbins - b0)
                qps = psp.tile([128, 512], f32, tag="mm")
                for i in range(nb):
                    b = b0 + i
                    lo = i * 128
                    nc.tensor.matmul(qps[:, lo:lo + 128], ones[:], bq[:],
                                     start=True, stop=False)
                    nc.tensor.matmul(qps[:, lo:lo + 128],
                                     qT[:, b * 128:(b + 1) * 128], Wq[:],
                                     start=False, stop=True)
                nc.vector.tensor_copy(qproj[:, b0 * 128:(b0 + nb) * 128],
                                      qps[:, 0:nb * 128])

            # ---- main edge loop ----------------------------------------
            for gi, g in enumerate(groups):
                gs0 = slotbase[g[0]]
                gslots = sum(D_list[b] for b in g)
                kve = kvep.tile([128, max_gslots, 256], f16, tag="kve")
                nc.gpsimd.dma_gather(
                    out_ap=kve[:, 0:gslots, :], in_ap=kvtab[:],
                    idxs_ap=didx[:, gs0 * 8:(gs0 + gslots) * 8],
                    num_idxs=gslots * 128, num_idxs_reg=gslots * 128,
                    elem_size=256, single_packet=False,
                )
                oUTps = opsp.tile([128, MAX_GROUP_BINS * 128], f32, tag="oUT")
                for j, b in enumerate(g):
                    D = D_list[b]
                    sb0 = slotbase[b] - gs0
                    kslice = kve[:, sb0:sb0 + D, 0:128]
                    vslice = kve[:, sb0:sb0 + D, 128:256]
                    # scores: prod + pairwise-add cascade (fp16, 2x mode)
                    prod = edgep.tile([128, maxD, 128], f16, tag="prod")
                    qb = qproj[:, b * 128:(b + 1) * 128].unsqueeze(1)
                    nc.vector.tensor_tensor(
                        prod[:, 0:D, :], kslice,
                        qb.broadcast_to([128, D, 128]),
                        op=mybir.AluOpType.mult)
                    casc = edgep.tile([128, maxD * 64], f16, tag="casc")
                    cv = casc[:].rearrange("p (s h d) -> p s h d",
                                           h=HEADS, d=16)
                    pv = prod[:].rearrange("p s (h d) -> p s h d", d=HD)
                    nc.vector.tensor_tensor(
                        cv[:, 0:D, :, 0:16], pv[:, 0:D, :, 0:16],
                        pv[:, 0:D, :, 16:32], op=mybir.AluOpType.add)
                    nc.vector.tensor_tensor(
                        cv[:, 0:D, :, 0:8], cv[:, 0:D, :, 0:8],
                        cv[:, 0:D, :, 8:16], op=mybir.AluOpType.add)
                    nc.vector.tensor_tensor(
                        cv[:, 0:D, :, 0:4], cv[:, 0:D, :, 0:4],
                        cv[:, 0:D, :, 4:8], op=mybir.AluOpType.add)
                    nc.vector.tensor_tensor(
                        cv[:, 0:D, :, 0:2], cv[:, 0:D, :, 0:2],
                        cv[:, 0:D, :, 2:4], op=mybir.AluOpType.add)
                    sm = smp.tile([128, maxD * HEADS], f16, tag="sm")
                    smv = sm[:].rearrange("p (s h) -> p s h", h=HEADS)
                    nc.vector.tensor_tensor(
                        smv[:, 0:D, :], cv[:, 0:D, :, 0],
                        cv[:, 0:D, :, 1], op=mybir.AluOpType.add)
                    # rbf * scores + mask
                    s0a = slotbase[b]
                    nc.vector.tensor_tensor(
                        smv[:, 0:D, :], smv[:, 0:D, :],
                        rbf_v[:, s0a:s0a + D, :], op=mybir.AluOpType.mult)
                    nc.vector.tensor_tensor(
                        smv[:, 0:D, :], smv[:, 0:D, :],
                        mask_v[:, s0a:s0a + D, :], op=mybir.AluOpType.add)
                    # exp
                    exps = smp.tile([128, maxD * HEADS], f16, tag="exps")
                    ev = exps[:].rearrange("p (s h) -> p s h", h=HEADS)
                    nc.scalar.activation(ev[:, 0:D, :], smv[:, 0:D, :],
                                         mybir.ActivationFunctionType.Exp)
                    # den = sum_s exps  (PE identity accumulation)
                    dps = dpsp.tile([128, HEADS], f32, tag="den")
                    for s in range(D):
                        nc.tensor.matmul(dps[:], ident[:], ev[:, s, :],
                                         start=(s == 0), stop=(s == D - 1))
                    den = smp.tile([128, HEADS], f32, tag="densb")
                    nc.vector.tensor_scalar_add(den[:], dps[:], 1e-8)
                    rec = smp.tile([128, HEADS], f32, tag="rec")
                    nc.vector.reciprocal(rec[:], den[:])
                    # probs (pair-duplicated for stride-1 wv multiply)
                    ex2 = smp.tile([128, maxD * HEADS * 2], f16, tag="ex2")
                    e2v = ex2[:].rearrange("p (s h two) -> p s h two", h=HEADS,
                                           two=2)
                    exn = smp.tile([128, maxD * HEADS], f16, tag="exn")
                    exnv = exn[:].rearrange("p (s h) -> p s h", h=HEADS)
                    nc.vector.tensor_tensor(
                        exnv[:, 0:D, :], ev[:, 0:D, :],
                        rec[:].unsqueeze(1).broadcast_to([128, D, HEADS]),
                        op=mybir.AluOpType.mult)
                    nc.vector.tensor_copy(e2v[:, 0:D, :, 0:1],
                                          exnv[:, 0:D, :].unsqueeze(3))
                    nc.vector.tensor_copy(e2v[:, 0:D, :, 1:2],
                                          exnv[:, 0:D, :].unsqueeze(3))
                    # wv = V * probs  (2x mode via pair duplication)
                    wv = edgep.tile([128, maxD, 128], f16, tag="wv")
                    wvv = wv[:].rearrange("p s (h d two) -> p s h d two",
                                          h=HEADS, d=16, two=2)
                    vv = vslice.rearrange("p s (h d two) -> p s h d two",
                                          h=HEADS, d=16, two=2)
                    e2b = e2v[:, 0:D, :, :].unsqueeze(3).broadcast_to(
                        [128, D, HEADS, 16, 2])
                    nc.vector.tensor_tensor(wvv[:, 0:D], vv, e2b,
                                            op=mybir.AluOpType.mult)
                    # outU^T accumulation: out[f, r] += wv_s[r, f]
                    for s in range(D):
                        nc.tensor.matmul(
                            oUTps[:, j * 128:(j + 1) * 128],
                            wv[:, s, :], ident[:],
                            start=(s == 0), stop=(s == D - 1))
                # group epilogue: drain outU^T, apply Wo, write out
                ng = len(g)
                oUT = finp.tile([128, MAX_GROUP_BINS * 128], f16, tag="oUTsb")
                nc.scalar.copy(oUT[:, 0:ng * 128], oUTps[:, 0:ng * 128])
                fps = fpsp.tile([128, MAX_GROUP_BINS * 128], f32, tag="fin")
                for j in range(ng):
                    nc.tensor.matmul(fps[:, j * 128:(j + 1) * 128],
                                     oUT[:, j * 128:(j + 1) * 128], Wo[:],
                                     start=True, stop=True)
                osb = finp.tile([128, MAX_GROUP_BINS * 128], f16, tag="osb")
                nc.scalar.copy(osb[:, 0:ng * 128], fps[:, 0:ng * 128])
                b0 = g[0]
                nc.sync.dma_start(
                    t_out[:, b0 * 128:(b0 + ng) * 128], osb[:, 0:ng * 128])

    nc.compile()
    return nc


def _wrap16(idx, n_slots):
    w = np.zeros((16, n_slots // 16), dtype=np.int16)
    w[:, :] = idx.astype(np.int16).reshape(n_slots // 16, 16).T
    return np.tile(w, (8, 1))


def kernel(**inputs):
    query = np.asarray(inputs["query"], np.float32)
    key_in = np.asarray(inputs["key_in"], np.float32)
    value_in = np.asarray(inputs["value_in"], np.float32)
    src = np.asarray(inputs["src"]).astype(np.int64)
    dst = np.asarray(inputs["dst"]).astype(np.int64)
    ea = np.asarray(inputs["edge_attr"], np.float32).reshape(-1)
    Wq = np.asarray(inputs["Wq"], np.float32)
    Wk = np.asarray(inputs["Wk"], np.float32)
    Wv = np.asarray(inputs["Wv"], np.float32)
    Wo = np.asarray(inputs["Wo"], np.float32)
    bq = np.asarray(inputs["bq"], np.float32)
    bk = np.asarray(inputs["bk"], np.float32)
    bv = np.asarray(inputs["bv"], np.float32)
    bo = np.asarray(inputs["bo"], np.float32)
    rbf_gamma = np.asarray(inputs["rbf_gamma"], np.float32)

    nq = query.shape[0]
    nkv = key_in.shape[0]
    E = src.shape[0]
    nkv_pad = ((nkv + 511) // 512) * 512

    gamma = np.maximum(rbf_gamma, np.float32(1e-8))
    rbf_all = (np.exp(-(gamma[None, :]) * (ea[:, None] ** 2))
               / np.float32(SCALE)).astype(np.float32)

    order = np.argsort(src, kind="stable")
    ssrc = src[order]
    sdst = dst[order]
    srbf = rbf_all[order]

    deg = np.bincount(src, minlength=nq).astype(np.int64)
    e_starts = np.zeros(nq + 1, dtype=np.int64)
    np.cumsum(deg, out=e_starts[1:])

    # degree-sorted rows (deg>0), dealt round-robin to cores
    rows_nz = np.nonzero(deg)[0]
    order_rows = rows_nz[np.argsort(deg[rows_nz], kind="stable")]
    core_rows = [order_rows[c::NCORES] for c in range(NCORES)]
    nrows_max = max(len(r) for r in core_rows)
    nbins = (nrows_max + 127) // 128

    # common bin template: D_b = max degree over all cores' rows in bin b
    D_list = []
    for b in range(nbins):
        mx = 1
        for c in range(NCORES):
            seg = core_rows[c][b * 128:(b + 1) * 128]
            if len(seg):
                mx = max(mx, int(deg[seg].max()))
        D_list.append(mx)
    slotbase = np.concatenate([[0], np.cumsum(D_list)]).astype(int)
    total_slots = int(slotbase[-1])
    if total_slots % 2:
        D_list[-1] += 1
        slotbase = np.concatenate([[0], np.cumsum(D_list)]).astype(int)
        total_slots = int(slotbase[-1])

    # gather groups: consecutive bins, caps on bins and slot-chunks
    groups = []
    cur = []
    cur_slots = 0
    for b in range(nbins):
        if cur and (len(cur) >= MAX_GROUP_BINS
                    or cur_slots + D_list[b] > MAX_GROUP_SLOTS):
            groups.append(tuple(cur))
            cur = []
            cur_slots = 0
        cur.append(b)
        cur_slots += D_list[b]
    if cur:
        groups.append(tuple(cur))

    key = (nkv_pad, nbins, tuple(D_list), tuple(groups), total_slots)
    if key not in _PROG_CACHE:
        _PROG_CACHE[key] = _build_program(nkv_pad, nbins, D_list, groups,
                                          total_slots)
    nc = _PROG_CACHE[key]

    kT_pad = np.zeros((128, nkv_pad), np.float16)
    kT_pad[:, :nkv] = key_in.T.astype(np.float16)
    vT_pad = np.zeros((128, nkv_pad), np.float16)
    vT_pad[:, :nkv] = value_in.T.astype(np.float16)
    ident_t = np.eye(128, dtype=np.float16)
    ones_t = np.ones((1, 128), np.float16)

    in_maps = []
    unpack = []
    for c in range(NCORES):
        rows_c = core_rows[c]
        qT = np.zeros((128, nbins * 128), np.float16)
        didx_cols = np.zeros((total_slots, 128), np.int64)
        rbf_cols = np.zeros((total_slots, 128, HEADS), np.float16)
        mask_cols = np.full((total_slots, 128, HEADS), np.float16(MASK_PAD))
        rows_glob = np.zeros(nbins * 128, np.int64) - 1

        for b in range(nbins):
            rows = rows_c[b * 128:(b + 1) * 128]
            nr = len(rows)
            if nr == 0:
                continue
            D = D_list[b]
            sb = slotbase[b]
            qT[:, b * 128:b * 128 + nr] = query[rows].T.astype(np.float16)
            rows_glob[b * 128:b * 128 + nr] = rows
            degs = deg[rows]
            e0 = e_starts[rows]
            sgrid = np.arange(D)[None, :]
            idx2d = e0[:, None] + sgrid
            valid = sgrid < degs[:, None]
            idx2d = np.where(valid, idx2d, 0)
            d2d = np.where(valid, sdst[idx2d], nkv)
            didx_cols[sb:sb + D, :nr] = d2d.T
            r2d = np.where(valid[:, :, None], srbf[idx2d], 0.0)
            rbf_cols[sb:sb + D, :nr] = r2d.transpose(1, 0, 2)
            m2d = np.where(valid, 0.0, MASK_PAD)
            mask_cols[sb:sb + D, :nr] = m2d.T[:, :, None]

        didx_flat = didx_cols.reshape(-1)
        in_maps.append({
            "qT": qT, "kT": kT_pad, "vT": vT_pad,
            "Wq": Wq.astype(np.float16), "Wk": Wk.astype(np.float16),
            "Wv": Wv.astype(np.float16), "Wo": Wo.astype(np.float16),
            "bq": bq.reshape(1, 128).astype(np.float16),
            "bk": bk.reshape(1, 128).astype(np.float16),
            "bv": bv.reshape(1, 128).astype(np.float16),
            "ones1": ones_t, "ident": ident_t,
            "rbfm": np.ascontiguousarray(
                rbf_cols.transpose(1, 0, 2)).reshape(128, -1),
            "mask": np.ascontiguousarray(
                mask_cols.transpose(1, 0, 2)).reshape(128, -1),
            "didx": _wrap16(didx_flat, total_slots * 128),
        })
        unpack.append(rows_glob)

    from concourse.bass_utils import run_bass_kernel_spmd
    g = globals()
    g["LAST_NC"] = nc
    g["LAST_INMAPS"] = in_maps
    res = run_bass_kernel_spmd(nc, in_maps, list(range(NCORES)),
                               trace=g.get("TRACE", False))
    g["LAST_RESULTS"] = res

    out = np.zeros((nq, HIDDEN), np.float32)
    for c in range(NCORES):
        o = np.asarray(res.results[c]["out"]).astype(np.float32)
        o = o.reshape(128, -1, 128).transpose(1, 0, 2).reshape(-1, 128)
        valid = unpack[c] >= 0
        out[unpack[c][valid]] = o[valid]
    out += bo[None, :]
    return out


# revision 7
# speedup vs baseline: 14.9129x; 8.6840x over previous
"""DistanceWeightedAttention Trainium2 kernel (8 NeuronCores, SPMD).

Strategy (degree-sorted row layout):
  - Rows (query nodes) with deg>0 are sorted by degree and dealt round-robin
    to the 8 cores, so every core sees an identical degree profile and the
    SPMD program (one bass module for all cores) has a common bin template.
  - A bin = 128 rows (partition dim) x D slots (free dim), D = max degree in
    the bin; degree sorting makes padding negligible. Edge (row j, slot s) of
    bin b sits at gather position (slotbase_b + s)*128 + j, so a single
    SWDGE gather of [K|V] fp16 rows (512B descriptors -- the cost-model
    sweet spot) lands kve[j, slot, 0:256] with partition = row.
  - Per bin, on-device:
      prod = K_e * q_row (DVE fp16, row broadcast over slots, 2x mode)
      scores = pairwise-add cascade 32->16->8->4->2->1 (all fp16 TT, 2x)
      scores = scores * rbf + mask  (pads get -80 -> exp == 0 in fp16)
      exps = ACT Exp; den = sum_s exps via PE identity-matmul accumulation
      rec = 1/(den+1e-8); exn = exps*rec; pair-duplicated exps2 so the
      wv multiply keeps a stride-1 last dim (DVE 2x mode)
      wv = V_e * probs; outU^T accumulated with matmul(lhsT=wv, rhs=I)
      out = outU^T.T @ Wo via one matmul per bin (bo added on host)
  - K/V/Q projections are computed on device in fp16 (biases as rank-1
    matmuls); kvtab ([K|V] per dst node) is staged in DRAM fp16.
  - Softmax uses the unstable form exp(s)/(sum exp(s)+1e-8); scores are O(1)
    here so this matches the reference's max(0,segmax) form to ~1e-7.
"""

import sys

import numpy as np

sys.path.insert(0, "/opt/trn_rl_repo")

HIDDEN = 128
HEADS = 4
HD = 32
SCALE = float(np.sqrt(HD))
NCORES = 8
MASK_PAD = -80.0
MAX_GROUP_SLOTS = 40      # slot-chunks per gather group cap (SBUF)
MAX_GROUP_BINS = 4

_PROG_CACHE = {}


def _build_program(nkv_pad, nbins, D_list, groups, total_slots):
    import concourse.bass as bass
    import concourse.bacc as bacc
    import concourse.tile as tile
    from concourse import mybir

    f32 = mybir.dt.float32
    f16 = mybir.dt.float16
    i16 = mybir.dt.int16
    nkv_tiles = nkv_pad // 128
    KSLAB = 16
    maxD = max(D_list)
    max_gslots = max(sum(D_list[b] for b in g) for g in groups)
    slotbase = np.concatenate([[0], np.cumsum(D_list)]).astype(int)

    nc = bacc.Bacc("TRN2", target_bir_lowering=False, debug=False,
                   num_devices=NCORES)

    t_qT = nc.dram_tensor("qT", [128, nbins * 128], f16, kind="ExternalInput")
    t_kT = nc.dram_tensor("kT", [128, nkv_pad], f16, kind="ExternalInput")
    t_vT = nc.dram_tensor("vT", [128, nkv_pad], f16, kind="ExternalInput")
    t_Wq = nc.dram_tensor("Wq", [128, 128], f16, kind="ExternalInput")
    t_Wk = nc.dram_tensor("Wk", [128, 128], f16, kind="ExternalInput")
    t_Wv = nc.dram_tensor("Wv", [128, 128], f16, kind="ExternalInput")
    t_Wo = nc.dram_tensor("Wo", [128, 128], f16, kind="ExternalInput")
    t_bq = nc.dram_tensor("bq", [1, 128], f16, kind="ExternalInput")
    t_bk = nc.dram_tensor("bk", [1, 128], f16, kind="ExternalInput")
    t_bv = nc.dram_tensor("bv", [1, 128], f16, kind="ExternalInput")
    t_ones = nc.dram_tensor("ones1", [1, 128], f16, kind="ExternalInput")
    t_ident = nc.dram_tensor("ident", [128, 128], f16, kind="ExternalInput")
    t_rbfm = nc.dram_tensor("rbfm", [128, total_slots * HEADS], f16,
                            kind="ExternalInput")
    t_mask = nc.dram_tensor("mask", [128, total_slots * HEADS], f16,
                            kind="ExternalInput")
    t_didx = nc.dram_tensor("didx", [128, total_slots * 8], i16,
                            kind="ExternalInput")
    t_out = nc.dram_tensor("out", [128, nbins * 128], f16,
                           kind="ExternalOutput")

    with tile.TileContext(nc) as tc:
        with (
            tc.tile_pool(name="const", bufs=1) as constp,
            tc.tile_pool(name="slab", bufs=2) as slabp,
            tc.tile_pool(name="work", bufs=3) as work,
            tc.tile_pool(name="kve", bufs=2) as kvep,
            tc.tile_pool(name="edge", bufs=3) as edgep,
            tc.tile_pool(name="sm", bufs=4) as smp,
            tc.tile_pool(name="fin", bufs=3) as finp,
            tc.tile_pool(name="ps", bufs=2, space="PSUM") as psp,
            tc.tile_pool(name="dps", bufs=2, space="PSUM") as dpsp,
            tc.tile_pool(name="ops", bufs=2, space="PSUM") as opsp,
            tc.tile_pool(name="fps", bufs=2, space="PSUM") as fpsp,
            tc.tile_pool(name="dram", bufs=1, space="DRAM") as dramp,
        ):
            Wq = constp.tile([128, 128], f16, tag="Wq")
            Wk = constp.tile([128, 128], f16, tag="Wk")
            Wv = constp.tile([128, 128], f16, tag="Wv")
            Wo = constp.tile([128, 128], f16, tag="Wo")
            bq = constp.tile([1, 128], f16, tag="bq")
            bk = constp.tile([1, 128], f16, tag="bk")
            bv = constp.tile([1, 128], f16, tag="bv")
            ones = constp.tile([1, 128], f16, tag="ones")
            ident = constp.tile([128, 128], f16, tag="ident")
            rbfm = constp.tile([128, total_slots * HEADS], f16, tag="rbfm")
            maskb = constp.tile([128, total_slots * HEADS], f16, tag="maskb")
            didx = constp.tile([128, total_slots * 8], i16, tag="didx")
            qT = constp.tile([128, nbins * 128], f16, tag="qT")
            qproj = constp.tile([128, nbins * 128], f16, tag="qproj")
            nc.sync.dma_start(Wq[:], t_Wq[:])
            nc.sync.dma_start(Wk[:], t_Wk[:])
            nc.sync.dma_start(Wv[:], t_Wv[:])
            nc.sync.dma_start(Wo[:], t_Wo[:])
            nc.sync.dma_start(bq[:], t_bq[:])
            nc.sync.dma_start(bk[:], t_bk[:])
            nc.sync.dma_start(bv[:], t_bv[:])
            nc.sync.dma_start(ones[:], t_ones[:])
            nc.sync.dma_start(ident[:], t_ident[:])
            nc.scalar.dma_start(rbfm[:], t_rbfm[:])
            nc.scalar.dma_start(maskb[:], t_mask[:])
            nc.scalar.dma_start(didx[:], t_didx[:])
            nc.sync.dma_start(qT[:], t_qT[:])
            rbf_v = rbfm[:].rearrange("p (s h) -> p s h", h=HEADS)
            mask_v = maskb[:].rearrange("p (s h) -> p s h", h=HEADS)

            kvtab = dramp.tile([nkv_pad, 256], f16, tag="kvtab")

            # ---- K/V projection -> kvtab fp16 ---------------------------
            for s0 in range(0, nkv_tiles, KSLAB):
                nt = min(KSLAB, nkv_tiles - s0)
                ksl = slabp.tile([128, KSLAB * 128], f16, tag="ksl")
                vsl = slabp.tile([128, KSLAB * 128], f16, tag="vsl")
                nc.sync.dma_start(ksl[:, 0:nt * 128],
                                  t_kT[:, s0 * 128:(s0 + nt) * 128])
                nc.scalar.dma_start(vsl[:, 0:nt * 128],
                                    t_vT[:, s0 * 128:(s0 + nt) * 128])
                for g0 in range(0, nt, 4):
                    ng = min(4, nt - g0)
                    kvsb = work.tile([128, 1024], f16, tag="kvsb")
                    for h0 in range(0, ng, 2):
                        kvps = psp.tile([128, 512], f32, tag="mm")
                        for i in range(2):
                            t = g0 + h0 + i
                            if t >= nt:
                                continue
                            lo = i * 256
                            nc.tensor.matmul(kvps[:, lo:lo + 128], ones[:],
                                             bk[:], start=True, stop=False)
                            nc.tensor.matmul(kvps[:, lo:lo + 128],
                                             ksl[:, t * 128:(t + 1) * 128],
                                             Wk[:], start=False, stop=True)
                            nc.tensor.matmul(kvps[:, lo + 128:lo + 256],
                                             ones[:], bv[:],
                                             start=True, stop=False)
                            nc.tensor.matmul(kvps[:, lo + 128:lo + 256],
                                             vsl[:, t * 128:(t + 1) * 128],
                                             Wv[:], start=False, stop=True)
                        nc.scalar.copy(kvsb[:, h0 * 256:h0 * 256 + 512],
                                       kvps[:])
                    nc.gpsimd.dma_start(
                        kvtab[(s0 + g0) * 128:(s0 + g0 + ng) * 128, :]
                        .rearrange("(t p) f -> p t f", p=128),
                        kvsb[:, 0:ng * 256].rearrange(
                            "p (t f) -> p t f", f=256))

            # ---- Q projection -> qproj fp16 (SBUF resident) -------------
            for b0 in range(0, nbins, 4):
                nb = min(4, nbins - b0)
                qps = psp.tile([128, 512], f32, tag="mm")
                for i in range(nb):
                    b = b0 + i
                    lo = i * 128
                    nc.tensor.matmul(qps[:, lo:lo + 128], ones[:], bq[:],
                                     start=True, stop=False)
                    nc.tensor.matmul(qps[:, lo:lo + 128],
                                     qT[:, b * 128:(b + 1) * 128], Wq[:],
                                     start=False, stop=True)
                nc.scalar.copy(qproj[:, b0 * 128:(b0 + nb) * 128],
                               qps[:, 0:nb * 128])

            # ---- main edge loop ----------------------------------------
            for gi, g in enumerate(groups):
                gs0 = slotbase[g[0]]
                gslots = sum(D_list[b] for b in g)
                kve = kvep.tile([128, max_gslots, 256], f16, tag="kve")
                nc.gpsimd.dma_gather(
                    out_ap=kve[:, 0:gslots, :], in_ap=kvtab[:],
                    idxs_ap=didx[:, gs0 * 8:(gs0 + gslots) * 8],
                    num_idxs=gslots * 128, num_idxs_reg=gslots * 128,
                    elem_size=256, single_packet=False,
                )
                oUTps = opsp.tile([128, MAX_GROUP_BINS * 128], f32, tag="oUT")
                GS = max_gslots
                # prod (per bin: q-row broadcast differs), group tiles
                prod = edgep.tile([128, GS, 128], f16, tag="prod")
                for b in g:
                    D = D_list[b]
                    sb0 = slotbase[b] - gs0
                    qb = qproj[:, b * 128:(b + 1) * 128].unsqueeze(1)
                    nc.vector.tensor_tensor(
                        prod[:, sb0:sb0 + D, :], kve[:, sb0:sb0 + D, 0:128],
                        qb.broadcast_to([128, D, 128]),
                        op=mybir.AluOpType.mult)
                # pairwise-add cascade over the whole group (fp16, 2x mode)
                casc = edgep.tile([128, GS * 64], f16, tag="casc")
                cv = casc[:].rearrange("p (s h d) -> p s h d", h=HEADS, d=16)
                pv = prod[:].rearrange("p s (h d) -> p s h d", d=HD)
                G_ = gslots
                nc.vector.tensor_tensor(
                    cv[:, 0:G_, :, 0:16], pv[:, 0:G_, :, 0:16],
                    pv[:, 0:G_, :, 16:32], op=mybir.AluOpType.add)
                nc.vector.tensor_tensor(
                    cv[:, 0:G_, :, 0:8], cv[:, 0:G_, :, 0:8],
                    cv[:, 0:G_, :, 8:16], op=mybir.AluOpType.add)
                nc.vector.tensor_tensor(
                    cv[:, 0:G_, :, 0:4], cv[:, 0:G_, :, 0:4],
                    cv[:, 0:G_, :, 4:8], op=mybir.AluOpType.add)
                nc.vector.tensor_tensor(
                    cv[:, 0:G_, :, 0:2], cv[:, 0:G_, :, 0:2],
                    cv[:, 0:G_, :, 2:4], op=mybir.AluOpType.add)
                sm = smp.tile([128, GS * HEADS], f16, tag="sm")
                smv = sm[:].rearrange("p (s h) -> p s h", h=HEADS)
                nc.vector.tensor_tensor(
                    smv[:, 0:G_, :], cv[:, 0:G_, :, 0],
                    cv[:, 0:G_, :, 1], op=mybir.AluOpType.add)
                # scores * rbf + mask (group)
                nc.vector.tensor_tensor(
                    smv[:, 0:G_, :], smv[:, 0:G_, :],
                    rbf_v[:, gs0:gs0 + G_, :], op=mybir.AluOpType.mult)
                nc.vector.tensor_tensor(
                    smv[:, 0:G_, :], smv[:, 0:G_, :],
                    mask_v[:, gs0:gs0 + G_, :], op=mybir.AluOpType.add)
                # exp (group)
                exps = smp.tile([128, GS * HEADS], f16, tag="exps")
                ev = exps[:].rearrange("p (s h) -> p s h", h=HEADS)
                nc.scalar.activation(ev[:, 0:G_, :], smv[:, 0:G_, :],
                                     mybir.ActivationFunctionType.Exp)
                # den per bin (PE identity accumulation) + normalize probs
                ex2 = smp.tile([128, GS * HEADS * 2], f16, tag="ex2")
                e2v = ex2[:].rearrange("p (s h two) -> p s h two", h=HEADS,
                                       two=2)
                for b in g:
                    D = D_list[b]
                    sb0 = slotbase[b] - gs0
                    dps = dpsp.tile([128, HEADS], f32, tag="den")
                    for s in range(D):
                        nc.tensor.matmul(dps[:], ident[:], ev[:, sb0 + s, :],
                                         start=(s == 0), stop=(s == D - 1))
                    den = smp.tile([128, HEADS], f32, tag="densb")
                    nc.vector.tensor_scalar_add(den[:], dps[:], 1e-8)
                    rec = smp.tile([128, HEADS], f32, tag="rec")
                    nc.vector.reciprocal(rec[:], den[:])
                    nc.vector.tensor_tensor(
                        e2v[:, sb0:sb0 + D, :, :],
                        ev[:, sb0:sb0 + D, :].unsqueeze(3).broadcast_to(
                            [128, D, HEADS, 2]),
                        rec[:].unsqueeze(1).unsqueeze(3).broadcast_to(
                            [128, D, HEADS, 2]),
                        op=mybir.AluOpType.mult)
                # wv = V * probs (group, 2x via pair duplication)
                wv = edgep.tile([128, GS, 128], f16, tag="wv")
                wvv = wv[:].rearrange("p s (h d two) -> p s h d two",
                                      h=HEADS, d=16, two=2)
                vv = kve[:, 0:G_, 128:256].rearrange(
                    "p s (h d two) -> p s h d two", h=HEADS, d=16, two=2)
                e2b = e2v[:, 0:G_, :, :].unsqueeze(3).broadcast_to(
                    [128, G_, HEADS, 16, 2])
                nc.vector.tensor_tensor(wvv[:, 0:G_], vv, e2b,
                                        op=mybir.AluOpType.mult)
                # outU^T accumulation: out[f, r] += wv_s[r, f]
                for j, b in enumerate(g):
                    D = D_list[b]
                    sb0 = slotbase[b] - gs0
                    for s in range(D):
                        nc.tensor.matmul(
                            oUTps[:, j * 128:(j + 1) * 128],
                            wv[:, sb0 + s, :], ident[:],
                            start=(s == 0), stop=(s == D - 1))
                # group epilogue: drain outU^T, apply Wo, write out
                ng = len(g)
                oUT = finp.tile([128, MAX_GROUP_BINS * 128], f16, tag="oUTsb")
                nc.scalar.copy(oUT[:, 0:ng * 128], oUTps[:, 0:ng * 128])
                fps = fpsp.tile([128, MAX_GROUP_BINS * 128], f32, tag="fin")
                for j in range(ng):
                    nc.tensor.matmul(fps[:, j * 128:(j + 1) * 128],
                                     oUT[:, j * 128:(j + 1) * 128], Wo[:],
                                     start=True, stop=True)
                osb = finp.tile([128, MAX_GROUP_BINS * 128], f16, tag="osb")
                nc.scalar.copy(osb[:, 0:ng * 128], fps[:, 0:ng * 128])
                b0 = g[0]
                nc.sync.dma_start(
                    t_out[:, b0 * 128:(b0 + ng) * 128], osb[:, 0:ng * 128])

    nc.compile()
    return nc


def _wrap16(idx, n_slots):
    w = np.zeros((16, n_slots // 16), dtype=np.int16)
    w[:, :] = idx.astype(np.int16).reshape(n_slots // 16, 16).T
    return np.tile(w, (8, 1))


def kernel(**inputs):
    query = np.asarray(inputs["query"], np.float32)
    key_in = np.asarray(inputs["key_in"], np.float32)
    value_in = np.asarray(inputs["value_in"], np.float32)
    src = np.asarray(inputs["src"]).astype(np.int64)
    dst = np.asarray(inputs["dst"]).astype(np.int64)
    ea = np.asarray(inputs["edge_attr"], np.float32).reshape(-1)
    Wq = np.asarray(inputs["Wq"], np.float32)
    Wk = np.asarray(inputs["Wk"], np.float32)
    Wv = np.asarray(inputs["Wv"], np.float32)
    Wo = np.asarray(inputs["Wo"], np.float32)
    bq = np.asarray(inputs["bq"], np.float32)
    bk = np.asarray(inputs["bk"], np.float32)
    bv = np.asarray(inputs["bv"], np.float32)
    bo = np.asarray(inputs["bo"], np.float32)
    rbf_gamma = np.asarray(inputs["rbf_gamma"], np.float32)

    nq = query.shape[0]
    nkv = key_in.shape[0]
    E = src.shape[0]
    nkv_pad = ((nkv + 511) // 512) * 512

    gamma = np.maximum(rbf_gamma, np.float32(1e-8))
    rbf_all = (np.exp(-(gamma[None, :]) * (ea[:, None] ** 2))
               / np.float32(SCALE)).astype(np.float32)

    order = np.argsort(src, kind="stable")
    ssrc = src[order]
    sdst = dst[order]
    srbf = rbf_all[order]

    deg = np.bincount(src, minlength=nq).astype(np.int64)
    e_starts = np.zeros(nq + 1, dtype=np.int64)
    np.cumsum(deg, out=e_starts[1:])

    # degree-sorted rows (deg>0), dealt round-robin to cores
    rows_nz = np.nonzero(deg)[0]
    order_rows = rows_nz[np.argsort(deg[rows_nz], kind="stable")]
    core_rows = [order_rows[c::NCORES] for c in range(NCORES)]
    nrows_max = max(len(r) for r in core_rows)
    nbins = (nrows_max + 127) // 128

    # common bin template: D_b = max degree over all cores' rows in bin b
    D_list = []
    for b in range(nbins):
        mx = 1
        for c in range(NCORES):
            seg = core_rows[c][b * 128:(b + 1) * 128]
            if len(seg):
                mx = max(mx, int(deg[seg].max()))
        D_list.append(mx)
    slotbase = np.concatenate([[0], np.cumsum(D_list)]).astype(int)
    total_slots = int(slotbase[-1])
    if total_slots % 2:
        D_list[-1] += 1
        slotbase = np.concatenate([[0], np.cumsum(D_list)]).astype(int)
        total_slots = int(slotbase[-1])

    # gather groups: consecutive bins, caps on bins and slot-chunks
    groups = []
    cur = []
    cur_slots = 0
    for b in range(nbins):
        if cur and (len(cur) >= MAX_GROUP_BINS
                    or cur_slots + D_list[b] > MAX_GROUP_SLOTS):
            groups.append(tuple(cur))
            cur = []
            cur_slots = 0
        cur.append(b)
        cur_slots += D_list[b]
    if cur:
        groups.append(tuple(cur))

    key = (nkv_pad, nbins, tuple(D_list), tuple(groups), total_slots)
    if key not in _PROG_CACHE:
        _PROG_CACHE[key] = _build_program(nkv_pad, nbins, D_list, groups,
                                          total_slots)
    nc = _PROG_CACHE[key]

    kT_pad = np.zeros((128, nkv_pad), np.float16)
    kT_pad[:, :nkv] = key_in.T.astype(np.float16)
    vT_pad = np.zeros((128, nkv_pad), np.float16)
    vT_pad[:, :nkv] = value_in.T.astype(np.float16)
    ident_t = np.eye(128, dtype=np.float16)
    ones_t = np.ones((1, 128), np.float16)

    in_maps = []
    unpack = []
    for c in range(NCORES):
        rows_c = core_rows[c]
        qT = np.zeros((128, nbins * 128), np.float16)
        didx_cols = np.zeros((total_slots, 128), np.int64)
        rbf_cols = np.zeros((total_slots, 128, HEADS), np.float16)
        mask_cols = np.full((total_slots, 128, HEADS), np.float16(MASK_PAD))
        rows_glob = np.zeros(nbins * 128, np.int64) - 1

        for b in range(nbins):
            rows = rows_c[b * 128:(b + 1) * 128]
            nr = len(rows)
            if nr == 0:
                continue
            D = D_list[b]
            sb = slotbase[b]
            qT[:, b * 128:b * 128 + nr] = query[rows].T.astype(np.float16)
            rows_glob[b * 128:b * 128 + nr] = rows
            degs = deg[rows]
            e0 = e_starts[rows]
            sgrid = np.arange(D)[None, :]
            idx2d = e0[:, None] + sgrid
            valid = sgrid < degs[:, None]
            idx2d = np.where(valid, idx2d, 0)
            d2d = np.where(valid, sdst[idx2d], nkv)
            didx_cols[sb:sb + D, :nr] = d2d.T
            r2d = np.where(valid[:, :, None], srbf[idx2d], 0.0)
            rbf_cols[sb:sb + D, :nr] = r2d.transpose(1, 0, 2)
            m2d = np.where(valid, 0.0, MASK_PAD)
            mask_cols[sb:sb + D, :nr] = m2d.T[:, :, None]

        didx_flat = didx_cols.reshape(-1)
        in_maps.append({
            "qT": qT, "kT": kT_pad, "vT": vT_pad,
            "Wq": Wq.astype(np.float16), "Wk": Wk.astype(np.float16),
            "Wv": Wv.astype(np.float16), "Wo": Wo.astype(np.float16),
            "bq": bq.reshape(1, 128).astype(np.float16),
            "bk": bk.reshape(1, 128).astype(np.float16),
            "bv": bv.reshape(1, 128).astype(np.float16),
            "ones1": ones_t, "ident": ident_t,
            "rbfm": np.ascontiguousarray(
                rbf_cols.transpose(1, 0, 2)).reshape(128, -1),
            "mask": np.ascontiguousarray(
                mask_cols.transpose(1, 0, 2)).reshape(128, -1),
            "didx": _wrap16(didx_flat, total_slots * 128),
        })
        unpack.append(rows_glob)

    from concourse.bass_utils import run_bass_kernel_spmd
    g = globals()
    g["LAST_NC"] = nc
    g["LAST_INMAPS"] = in_maps
    res = run_bass_kernel_spmd(nc, in_maps, list(range(NCORES)),
                               trace=g.get("TRACE", False))
    g["LAST_RESULTS"] = res

    out = np.zeros((nq, HIDDEN), np.float32)
    for c in range(NCORES):
        o = np.asarray(res.results[c]["out"]).astype(np.float32)
        o = o.reshape(128, -1, 128).transpose(1, 0, 2).reshape(-1, 128)
        valid = unpack[c] >= 0
        out[unpack[c][valid]] = o[valid]
    out += bo[None, :]
    return out
